# revision 12
# baseline (speedup 1.0000x reference)
"""Multi-head causal attention (B=2, S=2048, D=1024, H=16) on 8 TRN2 cores.

Sharding (Megatron-style): core c handles batch b = c//4, head-group
g = c%4 (4 heads, d' slice of 256). Each core computes its partial
out = ctx_g @ Wo[gslice] (no bias); host sums the 4 partials per batch
and adds the bias.

Device kernel dataflow (fp16 matmul operands, fp32 PSUM accumulation):
  qT/kT [d', S] and v via PE projections ->
  scores sT[k, q] per (head-pair, k-tile, q-block) (row-packed K=64
  matmul pairs) -> exp on ACT (psum->sbuf, fused 1/sqrt(hd) scale) ->
  causal triangle masking via GPSIMD affine_select in-place; fully
  masked regions are simply skipped by partial-width matmuls ->
  ctxT + softmax denominators accumulated on PE (ones columns
  interleaved in the v operand) -> 1/denom = exp(-ln(denom)) on ACT ->
  normalize on DVE -> out projection on PE.
"""

import numpy as np

import concourse.bass as bass
import concourse.mybir as mybir
import concourse.tile as tile
from concourse import bacc, bass_utils
from concourse.hw_specs import get_activation_tables

F32 = mybir.dt.float32
F16 = mybir.dt.float16
EXP = mybir.ActivationFunctionType.Exp
LN = mybir.ActivationFunctionType.Ln

B, S, D, H, HD = 2, 2048, 1024, 16, 64
NHL = 4          # local heads per core
DC = NHL * HD    # 256 local d'
NDT = D // 128   # 8 contraction tiles for projections
KT = 128         # k tile
NKT = S // KT    # 16
QB = 512         # q block
NQB = S // QB    # 4
SCALE = 1.0 / np.sqrt(HD)

# va free-layout per k-tile m: [v0 | J | v1 | v2 | J | v3], J = ones(64)
# (matmul weight APs allow only one free dim, so the ones blocks are
# interleaved to make every head a contiguous 128-col slice).
# Head h reads 128 cols at VA_OFF[h]; even heads are [v|J] (ctx psum rows
# 0:64, denom 64:128), odd heads [J|v] (denom 0:64, ctx 64:128).
VA_W = 384
VA_OFF = [0, 64, 192, 256]


def _pin_act_table(arch):
    """Steer Bacc's activation-table chooser to the one set containing both
    exp and ln, so interleaved Exp/Ln calls don't thrash ACT_TABLE_LOADs
    (measured 1.28us per reload)."""
    tabs = get_activation_tables(arch)
    keep = "natural_log_exp_and_others"
    if keep not in tabs:
        return
    for name, funcs in tabs.items():
        if name != keep:
            funcs.discard(EXP)
            funcs.discard(LN)


def build_nc():
    nc = bacc.Bacc("TRN2", target_bir_lowering=False, debug=False)
    xT = nc.dram_tensor("xT", [NDT, 128, S], F16, kind="ExternalInput")
    wq = nc.dram_tensor("wq", [NDT, 128, DC], F16, kind="ExternalInput")
    wk = nc.dram_tensor("wk", [NDT, 128, DC], F16, kind="ExternalInput")
    wv = nc.dram_tensor("wv", [NDT, 128, DC], F16, kind="ExternalInput")
    wo = nc.dram_tensor("wo", [2, 128, D], F16, kind="ExternalInput")
    out = nc.dram_tensor("out_p", [S, D], F32, kind="ExternalOutput")

    with tile.TileContext(nc) as tc:
        with (
            tc.tile_pool(name="xp", bufs=1) as xp,
            tc.tile_pool(name="wp", bufs=1) as wp,
            tc.tile_pool(name="qk", bufs=1) as qkp,
            tc.tile_pool(name="vap", bufs=1) as vap,
            tc.tile_pool(name="cnp", bufs=1) as cnp,
            tc.tile_pool(name="et", bufs=6) as etp,
            tc.tile_pool(name="rcp", bufs=3) as rcp,
            tc.tile_pool(name="ob", bufs=3) as obp,
            tc.tile_pool(name="pp", bufs=2, space="PSUM") as pp,
            tc.tile_pool(name="sp", bufs=2, space="PSUM") as sp,
            tc.tile_pool(name="cp", bufs=2, space="PSUM") as cp,
        ):
            # ---- loads (interleaved so projection chains start early) ----
            wq_sb = wp.tile([128, NDT, DC], F16, tag="wq")
            wk_sb = wp.tile([128, NDT, DC], F16, tag="wk")
            wv_sb = wp.tile([128, NDT, DC], F16, tag="wv")
            wo_sb = wp.tile([128, 2, D], F16, tag="wo")
            xt_sb = [xp.tile([128, S], F16, tag=f"x{t}", name=f"x{t}")
                     for t in range(NDT)]
            for t in range(NDT):
                nc.sync.dma_start(wq_sb[:, t, :], wq.ap()[t])
                nc.sync.dma_start(wk_sb[:, t, :], wk.ap()[t])
                nc.sync.dma_start(xt_sb[t][:], xT.ap()[t])
            for t in range(NDT):
                nc.sync.dma_start(wv_sb[:, t, :], wv.ap()[t])
            for t in range(2):
                nc.sync.dma_start(wo_sb[:, t, :], wo.ap()[t])

            # ---- constants; also preload the ACT table early ----
            va = vap.tile([128, NKT, VA_W], F16)
            nc.gpsimd.memset(va[:], 1.0)
            tri = wp.tile([128, 128], F16, tag="tri")
            nc.gpsimd.memset(tri[:], 1.0)
            # tri[k, q] = 1 if q >= k else 0
            nc.gpsimd.affine_select(
                out=tri[:], in_=tri[:], compare_op=mybir.AluOpType.is_ge,
                fill=0.0, base=0, pattern=[[1, 128]], channel_multiplier=-1)
            warmup = wp.tile([1, 8], F32, tag="wu")
            nc.vector.memset(warmup[:], 1.0)
            nc.scalar.activation(warmup[:], warmup[:], EXP, scale=0.001)

            # ---- projections ----
            qT = [qkp.tile([128, S], F16, tag=f"q{hp}", name=f"q{hp}")
                  for hp in range(2)]
            kTt = [qkp.tile([128, S], F16, tag=f"k{hp}", name=f"k{hp}")
                   for hp in range(2)]
            ctxn = [cnp.tile([128, S], F16, tag=f"c{t}", name=f"c{t}")
                    for t in range(2)]

            def proj_chain(hp, dst, w_sb, j):
                ps = pp.tile([128, QB], F32, tag="pp")
                for t in range(NDT):
                    nc.tensor.matmul(
                        ps[:], w_sb[:, t, 128 * hp:128 * (hp + 1)],
                        xt_sb[t][:, QB * j:QB * (j + 1)],
                        start=(t == 0), stop=(t == NDT - 1))
                nc.vector.tensor_copy(dst[:, QB * j:QB * (j + 1)], ps[:])

            def proj_qk(hp):
                for j in range(NQB):
                    proj_chain(hp, qT[hp], wq_sb, j)
                    proj_chain(hp, kTt[hp], wk_sb, j)

            def proj_v(lo, hi):
                for m in range(lo, hi):
                    ps = pp.tile([128, DC], F32, tag="pp")
                    for t in range(NDT):
                        nc.tensor.matmul(
                            ps[:], xt_sb[t][:, 128 * m:128 * (m + 1)],
                            wv_sb[:, t, :], start=(t == 0), stop=(t == NDT - 1))
                    # [v0|J|v1|v2|J|v3]: 3 contiguous copies
                    nc.vector.tensor_copy(va[:, m, 0:64], ps[:, 0:64])
                    nc.vector.tensor_copy(va[:, m, 128:256], ps[:, 64:192])
                    nc.vector.tensor_copy(va[:, m, 320:384], ps[:, 192:256])

            def attention_j(hp, j):
                    cpe = cp.tile([128, QB], F32, tag="cp")   # head 2hp
                    cpo = cp.tile([128, QB], F32, tag="cp")   # head 2hp+1
                    nkt_j = 4 * j + 4
                    for i in range(nkt_j):
                        c0 = max(0, 128 * (i - 4 * j))
                        spt = sp.tile([128, 2 * QB], F32, tag="sp")
                        for lh in range(2):
                            nc.tensor.matmul(
                                spt[:, QB * lh + c0:QB * (lh + 1)],
                                kTt[hp][64 * lh:64 * (lh + 1), 128 * i:128 * (i + 1)],
                                qT[hp][64 * lh:64 * (lh + 1), QB * j + c0:QB * (j + 1)],
                                start=True, stop=True)
                        et = etp.tile([128, 2 * QB], F16, tag="et")
                        if c0 > 0:
                            for lh in range(2):
                                nc.scalar.activation(
                                    et[:, QB * lh + c0:QB * (lh + 1)],
                                    spt[:, QB * lh + c0:QB * (lh + 1)],
                                    EXP, scale=float(SCALE))
                        else:
                            nc.scalar.activation(et[:], spt[:],
                                                 EXP, scale=float(SCALE))
                        if i >= 4 * j:  # diagonal: triangle mask in place
                            for lh in range(2):
                                sl = et[:, QB * lh + c0:QB * lh + c0 + 128]
                                nc.gpsimd.affine_select(
                                    out=sl, in_=sl,
                                    compare_op=mybir.AluOpType.is_ge, fill=0.0,
                                    base=0, pattern=[[1, 128]],
                                    channel_multiplier=-1)
                        for lh, cpt in ((0, cpe), (1, cpo)):
                            h = 2 * hp + lh
                            nc.tensor.matmul(
                                cpt[:, c0:QB],
                                va[:, i, VA_OFF[h]:VA_OFF[h] + 128],
                                et[:, QB * lh + c0:QB * (lh + 1)],
                                start=(i == 0), stop=(i == nkt_j - 1))
                    # normalize; even heads [v|J]: ctx rows 0:64, denom
                    # 64:128; odd heads [J|v]: denom 0:64, ctx 64:128.
                    # 1/denom = exp(-ln(denom)) on ACT, computed on all 128
                    # partitions (64-partition ACT ops run half-rate; the
                    # ctx-row lanes produce junk that is never read), then
                    # a shifted-in1 DVE mul (verified exact on HW).
                    for lh, cpt in ((0, cpe), (1, cpo)):
                        t1 = rcp.tile([128, QB], F32, tag="t1")
                        rc = rcp.tile([128, QB], F32, tag="rc")
                        cr = slice(64 * lh, 64 * lh + 64)        # ctx rows
                        dr = slice(64 - 64 * lh, 128 - 64 * lh)  # denom rows
                        nc.scalar.activation(t1[:], cpt[:], LN)
                        nc.scalar.activation(rc[:], t1[:], EXP, scale=-1.0)
                        nc.vector.tensor_mul(
                            ctxn[hp][cr, QB * j:QB * (j + 1)], cpt[cr, :],
                            rc[dr, :])

            def out_chunk(m):
                ot = obp.tile([128, D], F32, tag="ob")
                for o in range(2):
                    ps = pp.tile([128, QB], F32, tag="pp")
                    for t in range(2):
                        nc.tensor.matmul(
                            ps[:], ctxn[t][:, 128 * m:128 * (m + 1)],
                            wo_sb[:, t, QB * o:QB * (o + 1)],
                            start=(t == 0), stop=(t == 1))
                    nc.vector.tensor_copy(ot[:, QB * o:QB * (o + 1)], ps[:])
                nc.sync.dma_start(out.ap()[128 * m:128 * (m + 1), :], ot[:])

            proj_qk(0)
            # pair-0 attention starts as soon as its q/k blocks and the va
            # k-tiles it consumes exist; pair-1 projections and the
            # remaining v chains fill PE while ACT chews exp.
            for j in range(NQB):
                proj_v(4 * j, 4 * (j + 1))
                attention_j(0, j)
                proj_chain(1, qT[1], wq_sb, j)
                proj_chain(1, kTt[1], wk_sb, j)
            # pair-1 attention with out-proj chunks riding behind: chunk m
            # only needs q-block j = m // 4 of both pairs' ctxn.
            for j in range(NQB):
                attention_j(1, j)
                for m in range(4 * j, 4 * j + 4):
                    out_chunk(m)
    _pin_act_table(nc.m.arch)
    nc.compile()
    return nc


_NC = None


def _get_nc():
    global _NC
    if _NC is None:
        _NC = build_nc()
    return _NC


def make_in_maps(x, Wq, Wk, Wv, Wo):
    x = np.asarray(x, np.float32)
    Wq, Wk, Wv, Wo = (np.asarray(w, np.float32) for w in (Wq, Wk, Wv, Wo))
    in_maps = []
    for c in range(8):
        b, g = c // 4, c % 4
        sl = slice(DC * g, DC * (g + 1))
        in_maps.append({
            "xT": np.ascontiguousarray(x[b].T).astype(np.float16)
                    .reshape(NDT, 128, S),
            "wq": np.ascontiguousarray(Wq[:, sl]).astype(np.float16)
                    .reshape(NDT, 128, DC),
            "wk": np.ascontiguousarray(Wk[:, sl]).astype(np.float16)
                    .reshape(NDT, 128, DC),
            "wv": np.ascontiguousarray(Wv[:, sl]).astype(np.float16)
                    .reshape(NDT, 128, DC),
            "wo": np.ascontiguousarray(Wo[sl, :]).astype(np.float16)
                    .reshape(2, 128, D),
        })
    return in_maps


def kernel(x, Wq, Wk, Wv, Wo, bo, _trace=False, _trace_cores=None):
    nc = _get_nc()
    in_maps = make_in_maps(x, Wq, Wk, Wv, Wo)
    res = bass_utils.run_bass_kernel_spmd(
        nc, in_maps, core_ids=list(range(8)), trace=_trace,
        trace_cores=_trace_cores)
    bo = np.asarray(bo, np.float32)
    out = np.empty((B, S, D), np.float32)
    for b in range(B):
        acc = res.results[4 * b]["out_p"].astype(np.float32).copy()
        for g in range(1, 4):
            acc += res.results[4 * b + g]["out_p"]
        out[b] = acc + bo
    kernel.last_results = res
    return out
